# revision 11
# baseline (speedup 1.0000x reference)
"""Trainium2 Bass kernel for nn_ContextualAttention.

Per sample b (one per NeuronCore):
    X   = foreground[b]               # [256, 4096]  (channels x pixels)
    K   = (X + eps).T, L2-normalized rows          # [4096, 256]
    S   = K @ X                        # [4096(k), 4096(p)] scores
    A   = softmax(S, axis=k)
    out = K.T @ A                      # [256, 4096]

v2 design (bf16 matmuls, wide ACT exps, Z on DVE):
    All matmul operands are bf16 (rel-err budget 2e-2; measured ~5e-3).
    X16   [128, 2, HW]   bf16 channels on partitions (2 chunks of 128)
    XT16  [128, KT, 256] bf16 k on partitions, via PE transpose of X16
    Khat16 = XT16 * recip_n[k]        # mm2 weights (row-normalized K)
    recip_n = exp(-0.5*ln(n2)) so Ln/Exp share one ACT table set.

    Main loop over 4 pixel tiles of PT=1024, k chunks of 128:
      s_ps  [128, 1024] f32 PSUM (2 banks) <- 4 bf16 matmuls (2 cc x 2 half)
      e16   [128, 1024] bf16 = ACT Exp(s_ps * recip_n[k])  (one wide
            ACTIVATE per k-chunk amortizes the ~352-cycle ACT overhead)
      out_ps[cc] [128, 1024] += Khat16.T @ e16   (4 bf16 matmuls, PSUM accum)
      zacc  [128, 1024] bf16 += e16 on DVE (2x packed mode) - the softmax
            denominator, replicated across partitions by construction? No:
            zacc[q, p] accumulates E[k,p] only for k == q (mod 128) summed
            over chunks; the final cross-partition reduction is done by a
            single ones-matmul per pixel tile (32x cheaper than per-chunk).
      PSUM: 2x2 banks s_ps (double buffered) + 2x2 banks out_ps = 8 banks.
    Epilogue per pixel tile: Z = ones^T @ zacc16 (1 matmul), rz =
    reciprocal_approx_fast(Z), out = out_ps * rz on DVE, DMA to DRAM.

eps=1e-7 is dropped: its effect on the output is O(1e-7) relative, far
below matmul precision.
"""

import numpy as np
from contextlib import ExitStack

import concourse.bass as bass
import concourse.tile as tile
from concourse import mybir
from concourse.bass_utils import run_bass_kernel_spmd
from concourse.masks import make_identity

F32 = mybir.dt.float32
BF16 = mybir.dt.bfloat16
FP8 = mybir.dt.float8e4
AF = mybir.ActivationFunctionType
ALU = mybir.AluOpType

CH = 256     # channels
P = 128      # partitions
PT = 1024    # pixel-tile width (2 psum banks)
HB = 512     # matmul moving-dim half (1 psum bank)
N_CORES = 8


def _emit(tc: "tile.TileContext", x: bass.AP, out: bass.AP, hw: int):
    nc = tc.nc
    CC = CH // P          # channel chunks (2)
    KT = hw // P          # k tiles (32)
    NPT = hw // PT        # pixel tiles (4)

    with ExitStack() as ctx:
        const = ctx.enter_context(tc.tile_pool(name="const", bufs=1))
        sb = ctx.enter_context(tc.tile_pool(name="sb", bufs=1))

        X32 = sb.tile([P, CC, hw], F32, tag="X32")
        X16 = sb.tile([P, CC, hw], BF16, tag="X16")
        X8 = sb.tile([P, CC, hw], FP8, tag="X8")
        XT16 = sb.tile([P, KT, CH], BF16, tag="XT16")
        Khat = sb.tile([P, KT, CH], BF16, tag="Khat")
        n2 = sb.tile([P, KT], F32, tag="n2")
        lnn2 = sb.tile([P, KT], F32, tag="lnn2")
        recip_n = sb.tile([P, KT], F32, tag="recip_n")

        ident = const.tile([P, P], F32, tag="ident")
        ident16 = const.tile([P, P], BF16, tag="ident16")
        ones16 = const.tile([P, P], BF16, tag="ones16")
        onesf = const.tile([P, P], F32, tag="onesf")
        make_identity(nc, ident)
        nc.vector.memset(onesf, 1.0)
        with nc.allow_low_precision(reason="bf16 matmul operand prep"):
            nc.vector.tensor_copy(ident16, ident)
            nc.vector.tensor_copy(ones16, onesf)

        # ---- load X: [256, hw] -> [128, cc, hw]; cast to bf16 as chunks
        # arrive. First slices small so the first transposes start ASAP.
        bounds = [0, hw // 16, hw // 4, hw // 2, 3 * hw // 4, hw]
        for lo, hi in zip(bounds, bounds[1:]):
            for cc in range(CC):
                nc.sync.dma_start(
                    out=X32[:, cc, lo:hi],
                    in_=x[cc * P:(cc + 1) * P, lo:hi],
                )
                with nc.allow_low_precision(reason="bf16/fp8 matmul operand prep"):
                    nc.vector.tensor_copy(X16[:, cc, lo:hi], X32[:, cc, lo:hi])
                    nc.vector.tensor_copy(X8[:, cc, lo:hi], X32[:, cc, lo:hi])

        # ---- setup: transpose X16 -> XT16; n2 = row sumsq (from f32 psum);
        # recip_n = 1/sqrt(n2) via exp(-0.5*ln(n2)) (Ln+Exp share one ACT
        # table set, unlike Sqrt); Khat = XT16 * recip_n.
        with tc.tile_pool(name="tpsum", bufs=3, space="PSUM") as tpsum, \
             tc.tile_pool(name="tsq", bufs=2) as tsq:
            for kt in range(KT):
                # full-bank (2KB) bf16 tile keeps psum bank alignment; only
                # the first 256 columns are used.
                pt_ = tpsum.tile([P, PT], BF16, tag="t")
                for cc in range(CC):
                    nc.tensor.transpose(
                        pt_[:, cc * P:(cc + 1) * P],
                        X16[:, cc, kt * P:(kt + 1) * P],
                        ident16,
                    )
                nc.vector.tensor_copy(XT16[:, kt, :], pt_[:, :CH])
                sq = tsq.tile([P, CH], F32, tag="sq")
                nc.scalar.activation(
                    sq,
                    pt_[:, :CH],
                    AF.Square,
                    accum_out=n2[:, kt:kt + 1],
                )
            # recip_n = exp(-0.5 * ln(n2))
            nc.scalar.activation(lnn2, n2, AF.Ln)
            nc.scalar.activation(recip_n, lnn2, AF.Exp, scale=-0.5)
            with nc.allow_low_precision(reason="bf16 matmul operand prep"):
                for kt in range(KT):
                    nc.vector.tensor_scalar_mul(
                        out=Khat[:, kt, :],
                        in0=XT16[:, kt, :],
                        scalar1=recip_n[:, kt:kt + 1],
                    )

        # ---- main: per pixel-tile flash attention ----
        with tc.tile_pool(name="ps", bufs=2, space="PSUM") as ps_pool, \
             tc.tile_pool(name="acc", bufs=2, space="PSUM") as acc_pool, \
             tc.tile_pool(name="ework", bufs=8) as e_pool, \
             tc.tile_pool(name="owork", bufs=4) as o_pool, \
             tc.tile_pool(name="zwork", bufs=2) as z_pool, \
             tc.tile_pool(name="zfwork", bufs=4) as zf_pool:
            for pt in range(NPT):
                out_ps = [
                    acc_pool.tile([P, PT], F32, tag="acc", name=f"out_ps{cc}")
                    for cc in range(CC)
                ]
                zacc = z_pool.tile([P, PT], BF16, tag="z")

                def mm2_group(kc, e16):
                    # out[c, p] += Khat[k, c].T @ E
                    for cc in range(CC):
                        for h in range(PT // HB):
                            nc.tensor.matmul(
                                out_ps[cc][:, h * HB:(h + 1) * HB],
                                lhsT=Khat[:, kc, cc * P:(cc + 1) * P],
                                rhs=e16[:, h * HB:(h + 1) * HB],
                                start=(kc == 0),
                                stop=(kc == KT - 1),
                            )
                    # partial softmax denominator on DVE (2x bf16 mode);
                    # cross-partition sum happens once per pixel tile below.
                    with nc.allow_low_precision(reason="bf16 denominator"):
                        if kc == 0:
                            nc.vector.tensor_copy(zacc, e16)
                        else:
                            nc.vector.tensor_add(zacc, zacc, e16)

                # Software-pipelined: mm2 for k-chunk kc issues after mm1 of
                # kc+1, giving each exp a full k-step of slack on ACT.
                pending = None
                for kc in range(KT):
                    # scores[k, p] = sum_c X[c, k] * X[c, p]
                    # DoubleRow fp8: the [128, cc=2, hw] blocked layout IS a
                    # valid contraction pairing (channel c paired with c+128
                    # in the same PE cell), so one matmul per 512-pixel half
                    # does the full 256-deep contraction at 2 MACs/cell/cycle.
                    s_ps = ps_pool.tile([P, PT], F32, tag="ps")
                    for h in range(PT // HB):
                        nc.tensor.matmul(
                            s_ps[:, h * HB:(h + 1) * HB],
                            lhsT=X8[:, :, kc * P:(kc + 1) * P],
                            rhs=X8[:, :, pt * PT + h * HB:
                                   pt * PT + (h + 1) * HB],
                            start=True,
                            stop=True,
                            perf_mode=mybir.MatmulPerfMode.DoubleRow,
                        )
                    # E = exp(recip_n[k] * s), one wide ACTIVATE
                    e16 = e_pool.tile([P, PT], BF16, tag="e")
                    with nc.allow_low_precision(reason="bf16 attention"):
                        nc.scalar.activation(
                            e16, s_ps, AF.Exp, scale=recip_n[:, kc:kc + 1],
                        )
                    if pending is not None:
                        mm2_group(*pending)
                    pending = (kc, e16)
                mm2_group(*pending)

                # Epilogue: Z[p] = sum_k E[k,p] via one ones-matmul over the
                # partition-partial zacc; rz = 1/Z; out = out_ps * rz.
                z_ps = ps_pool.tile([P, PT], F32, tag="ps", name="z_ps")
                for h in range(PT // HB):
                    nc.tensor.matmul(
                        z_ps[:, h * HB:(h + 1) * HB],
                        lhsT=ones16,
                        rhs=zacc[:, h * HB:(h + 1) * HB],
                        start=True,
                        stop=True,
                    )
                # rz = 1/Z as exp(-ln(Z)) on ACT: Ln/Exp live in the same
                # ACT table set as the softmax Exp (no table switch), and
                # this keeps the slow DVE reciprocal off the critical path.
                lnz = zf_pool.tile([P, PT], F32, tag="lnz")
                nc.scalar.activation(lnz, z_ps, AF.Ln)
                rz = zf_pool.tile([P, PT], F32, tag="rz")
                nc.scalar.activation(rz, lnz, AF.Exp, scale=-1.0)
                for cc in range(CC):
                    o_sb = o_pool.tile([P, PT], F32, tag="o", name=f"o{cc}")
                    nc.vector.tensor_mul(o_sb, out_ps[cc], rz)
                    nc.sync.dma_start(
                        out=out[cc * P:(cc + 1) * P, pt * PT:(pt + 1) * PT],
                        in_=o_sb,
                    )


def _legalize_single_wait(nc: bass.Bass) -> None:
    """The walrus build in this container accepts at most ONE sync-wait per
    instruction ("Too many sync wait commands"); Tile emits instructions with
    one wait per outstanding producer. Hoist extra waits onto injected
    same-engine NOPs placed immediately before the instruction — identical
    blocking semantics, one wait each."""
    for fn in nc.m.functions:
        for bb in fn.blocks:
            new = []
            changed = False
            for inst in bb.instructions:
                if (
                    isinstance(inst, mybir.InstISA)
                    and inst.engine == mybir.EngineType.Pool
                ):
                    # Tail-of-kernel semaphore RANGE_CLEAR on GpSimd; this
                    # walrus build rejects its encoding ("ISA wrong length").
                    # Semaphores are re-initialized by the runtime at
                    # execution start, so the in-kernel clear is redundant.
                    # (DVE InstISA ops — e.g. tensor_tensor_reduce — are real
                    # compute and must be kept.)
                    changed = True
                    continue
                si = inst.sync_info
                if si is not None and si.on_wait is not None and len(si.on_wait) > 1:
                    waits = list(si.on_wait)
                    for j, w in enumerate(waits[:-1]):
                        nop = mybir.InstNoOp(
                            name=f"{inst.name}-xw{j}",
                            engine=inst.engine,
                            sync_info=mybir.SyncInfo(on_wait=[w], on_update=[]),
                            bass_nofuse=True,
                        )
                        new.append(nop)
                    si.on_wait = [waits[-1]]
                    changed = True
                new.append(inst)
            if changed:
                bb.instructions = new


def build_nc(hw: int = 4096, legalize: bool = True) -> bass.Bass:
    nc = bass.Bass()
    x = nc.dram_tensor("x", [CH, hw], F32, kind="ExternalInput")
    out = nc.dram_tensor("out", [CH, hw], F32, kind="ExternalOutput")
    with tile.TileContext(nc) as tc:
        _emit(tc, x[:], out[:], hw)
    if legalize:
        _legalize_single_wait(nc)
    return nc


_nc_cache: dict = {}


def kernel(foreground: np.ndarray) -> np.ndarray:
    fg = np.ascontiguousarray(np.asarray(foreground, dtype=np.float32))
    bs, ch, h, w = fg.shape
    assert bs == N_CORES and ch == CH
    hw = h * w
    if hw not in _nc_cache:
        _nc_cache[hw] = build_nc(hw)
    nc = _nc_cache[hw]
    in_maps = [{"x": fg[i].reshape(ch, hw)} for i in range(bs)]
    res = run_bass_kernel_spmd(nc, in_maps, core_ids=list(range(bs)))
    return np.stack(
        [np.asarray(res.results[i]["out"]).reshape(ch, h, w) for i in range(bs)]
    )
